# revision 8
# baseline (speedup 1.0000x reference)
"""Trainium2 Bass kernel for ChunkedLocalSelfAttention.

Module: x[B,C,H,W] -> qkv proj -> 8-head local-window attention (17x17
spatial window) -> out proj -> +residual -> 1x1 conv -> relu.
B,C,H,W = 4,256,48,48; N = 2304 tokens per image; head dim 32.

Sharding: 8 cores = 4 batch images x 2 query-row-halves (24 rows each).
Each core computes the full pipeline for its half-image: attention output
rows only depend on +-8 image rows, so cores need no communication; the
row halo is covered by computing k/v for a 32-row band.

On-core design (scores kept TRANSPOSED: keys on partitions, queries free):
  - qk projection: qkT [512, 2304] = WqkT.T @ xT, bf16
  - v in [token, channel] layout: v = xT.T @ WvT for the 32-row band
  - per query-tile (8 image rows = 384 queries) and head-group (4 heads):
      for each of 9 key-chunks (128 contiguous tokens of the 24-row region):
        scoresT[k=128, h*512+q] via 4 row-packed K=32 matmuls
        exp on ScalarE (scale=1/sqrt(32) fused; logits are tiny so no
        max-subtraction is needed), multiplicative binary window mask on
        VectorE (mask precomputed on host, shared across heads)
        PV += v_chunk.T @ masked  (4 col-packed matmuls, M=32)
        sums += ones.T @ masked   (4 col-packed matmuls, M=32, the ones
                                   lhsT replicates each head's sum into
                                   its 32-partition strip)
      oT = PV * reciprocal(sums) -> bf16
  - out proj, +residual(+out_b folded on host), 1x1 conv, relu+conv_b.
"""

import sys

for _p in ("/opt/trn_rl_repo",):
    if _p not in sys.path:
        sys.path.insert(0, _p)

import math

import ml_dtypes
import numpy as np

B, C, H, W = 4, 256, 48, 48
N = H * W
HEADS, HD, HALF = 8, 32, 8
NCORES = 8
ROWS_HALF = H // 2          # 24 query rows per core
NQ = ROWS_HALF * W          # 1152 queries per core
REG_ROWS = 24               # key-region rows per query tile
NK = REG_ROWS * W           # 1152 keys per region = 9 chunks of 128
NCHUNK = NK // 128          # 9
BAND_ROWS = 32              # k/v row band per core (24 + 8 halo)
QT = 384                    # queries per tile (8 image rows)
SCALE = 1.0 / math.sqrt(HD)

bf16 = ml_dtypes.bfloat16

_PROG = None


def _build_program():
    import concourse.bass as bass
    import concourse.mybir as mybir
    import concourse.tile as tile
    from concourse import bacc

    f32 = mybir.dt.float32
    bft = mybir.dt.bfloat16
    AF = mybir.ActivationFunctionType
    OP = mybir.AluOpType

    nc = bacc.Bacc(
        "TRN2", target_bir_lowering=False, debug=False, num_devices=NCORES
    )

    def din(name, shape, dt=bft):
        return nc.dram_tensor(name, shape, dt, kind="ExternalInput").ap()

    xt_d = din("xT", [C, N])
    xres_d = din("xres", [C, NQ], f32)
    wqk_d = din("wqkT", [C, 2 * C])
    wv_d = din("wvT", [C, C])
    wo_d = din("woT", [C, C])
    wc_d = din("wcT", [C, C])
    bqk_d = din("bqk", [128, 4], f32)
    bv_d = din("bvrep", [128, C], f32)
    bc_d = din("bcrep", [128, 2], f32)
    mask_d = din("masks", [3 * NCHUNK, 128, QT])
    out_d = nc.dram_tensor("out", [C, NQ], f32, kind="ExternalOutput").ap()

    # SPMD trick: one program must serve both row-halves, but the two
    # halves have different boundary clamping. The host ships each core's
    # image ROLLED along rows (roll = 8 - 24*half) so the core's 24 query
    # rows always sit at rolled rows [8, 32). Key regions are then always
    # rolled rows [8*qt, 8*qt+24); rows that wrapped around carry far-away
    # true rows and are killed by the (true-coordinate) window mask.

    with tile.TileContext(nc) as tc:
        import contextlib

        ctx = contextlib.ExitStack()
        with ctx:
            cpool = ctx.enter_context(tc.tile_pool(name="const", bufs=1))
            qkpool = ctx.enter_context(tc.tile_pool(name="qk", bufs=1))
            vpool = ctx.enter_context(tc.tile_pool(name="v", bufs=1))
            mpool = ctx.enter_context(tc.tile_pool(name="mask", bufs=3))
            epool = ctx.enter_context(tc.tile_pool(name="exp", bufs=2))
            apool = ctx.enter_context(tc.tile_pool(name="attn", bufs=2))
            rpool = ctx.enter_context(tc.tile_pool(name="recip", bufs=2))
            opool = ctx.enter_context(tc.tile_pool(name="outb", bufs=3))
            psA = ctx.enter_context(
                tc.tile_pool(name="psA", bufs=1, space="PSUM")
            )
            psB = ctx.enter_context(
                tc.tile_pool(name="psB", bufs=4, space="PSUM")
            )

            # ---- constants / inputs to SBUF ----
            xt = [cpool.tile([128, N], bft, tag=f"xt{t}", name=f"xt{t}") for t in range(2)]
            for t in range(2):
                nc.sync.dma_start(xt[t][:], xt_d[128 * t : 128 * t + 128, :])
            wqk = [cpool.tile([128, 2 * C], bft, tag=f"wqk{t}", name=f"wqk{t}") for t in range(2)]
            wv = [cpool.tile([128, C], bft, tag=f"wv{t}", name=f"wv{t}") for t in range(2)]
            wo = [cpool.tile([128, C], bft, tag=f"wo{t}", name=f"wo{t}") for t in range(2)]
            wc = [cpool.tile([128, C], bft, tag=f"wc{t}", name=f"wc{t}") for t in range(2)]
            for t in range(2):
                sl = slice(128 * t, 128 * t + 128)
                nc.sync.dma_start(wqk[t][:], wqk_d[sl, :])
                nc.sync.dma_start(wv[t][:], wv_d[sl, :])
                nc.sync.dma_start(wo[t][:], wo_d[sl, :])
                nc.sync.dma_start(wc[t][:], wc_d[sl, :])
            bqk = cpool.tile([128, 4], f32, tag="bqk")
            bvr = cpool.tile([128, C], f32, tag="bvr")
            bcr = cpool.tile([128, 2], f32, tag="bcr")
            nc.sync.dma_start(bqk[:], bqk_d[:])
            nc.sync.dma_start(bvr[:], bv_d[:])
            nc.sync.dma_start(bcr[:], bc_d[:])
            xres = [cpool.tile([128, NQ], f32, tag=f"xres{t}", name=f"xres{t}") for t in range(2)]
            for t in range(2):
                nc.sync.dma_start(xres[t][:], xres_d[128 * t : 128 * t + 128, :])
            ones = cpool.tile([128, 32], bft, tag="ones")
            nc.vector.memset(ones[:], 1.0)

            # ---- phase 1: qk projection  qkT[512, N] bf16 ----
            qk = [qkpool.tile([128, N], bft, tag=f"qk{i}", name=f"qk{i}") for i in range(4)]
            NT = [(i * 512, min(512, N - i * 512)) for i in range(5)]
            for qc in range(4):
                for n0, nw in NT:
                    ps = psB.tile([128, 512], f32, tag="ps", name="ps")
                    for cc in range(2):
                        nc.tensor.matmul(
                            ps[:, :nw],
                            lhsT=wqk[cc][:, 128 * qc : 128 * qc + 128],
                            rhs=xt[cc][:, n0 : n0 + nw],
                            start=(cc == 0),
                            stop=(cc == 1),
                        )
                    nc.vector.tensor_scalar_add(
                        qk[qc][:, n0 : n0 + nw], ps[:, :nw], bqk[:, qc : qc + 1]
                    )

            # ---- phase 2: v band, token-major ----
            # rolled layout: regions span rolled rows [0, 40) -> 15 tiles.
            vt = [vpool.tile([128, C], bft, tag=f"v{i}", name=f"v{i}") for i in range(15)]
            for i in range(15):
                n0 = 128 * i
                ps = psB.tile([128, 512], f32, tag="ps", name="ps")
                for cc in range(2):
                    nc.tensor.matmul(
                        ps[:, :C],
                        lhsT=xt[cc][:, n0 : n0 + 128],
                        rhs=wv[cc][:],
                        start=(cc == 0),
                        stop=(cc == 1),
                    )
                nc.vector.tensor_add(vt[i][:], ps[:, :C], bvr[:])

            # ---- phase 3: attention ----
            # rolled coords: query rows [8, 32): qtile qt rows r0 = 8+8*qt,
            # region rows rs = 8*qt, region tokens [rs*48, rs*48+1152).
            oT = [cpool.tile([128, NQ], bft, tag=f"oT{g}", name=f"oT{g}") for g in range(2)]
            for qt in range(3):
                r0 = 8 + 8 * qt
                rs = 8 * qt
                q0 = r0 * W
                for g in range(2):
                    # col-packed accumulation: 4 heads share each bank, so
                    # start=True (bank-wide has_written clear) is unusable;
                    # zero the banks and accumulate from the first matmul.
                    pv = psB.tile([128, QT], f32, tag="ps", name="ps")
                    sm = psB.tile([128, QT], f32, tag="ps", name="ps")
                    nc.vector.memset(pv[:], 0.0)
                    nc.vector.memset(sm[:], 0.0)
                    for ck in range(NCHUNK):
                        kof = rs * W + 128 * ck
                        sc = psA.tile([128, 2048], f32, tag="sc", name="sc")
                        for hh in range(4):
                            h = 4 * g + hh
                            qtile_idx, krow = h // 4, 32 * (h % 4)
                            nc.tensor.matmul(
                                sc[:, 512 * hh : 512 * hh + QT],
                                lhsT=qk[2 + qtile_idx][
                                    krow : krow + 32, kof : kof + 128
                                ],
                                rhs=qk[qtile_idx][krow : krow + 32, q0 : q0 + QT],
                                start=True,
                                stop=True,
                                tile_position=(krow, 0),
                            )
                        ex = epool.tile([128, 4 * QT], bft, tag="ex", name="ex")
                        sc_v = sc[:].rearrange("p (h q) -> p h q", q=512)[
                            :, :, :QT
                        ]
                        ex_v = ex[:].rearrange("p (h q) -> p h q", q=QT)
                        nc.scalar.activation(ex_v, sc_v, AF.Exp, scale=SCALE)
                        mk = mpool.tile([128, QT], bft, tag="mk", name="mk")
                        nc.sync.dma_start(mk[:], mask_d[qt * NCHUNK + ck])
                        ma = apool.tile([128, 4 * QT], bft, tag="ma", name="ma")
                        ma_v = ma[:].rearrange("p (h q) -> p h q", q=QT)
                        nc.vector.tensor_mul(
                            ma_v, ex_v, mk[:, None, :].broadcast_to([128, 4, QT])
                        )
                        vi = vt[(rs * W + 128 * ck) // 128]
                        for hh in range(4):
                            h = 4 * g + hh
                            nc.tensor.matmul(
                                pv[32 * hh : 32 * hh + 32, :],
                                lhsT=vi[:, 32 * h : 32 * h + 32],
                                rhs=ma[:, QT * hh : QT * hh + QT],
                                start=False,
                                stop=(ck == NCHUNK - 1 and hh == 3),
                                skip_group_check=True,
                                tile_position=(0, 32 * hh),
                            )
                            nc.tensor.matmul(
                                sm[32 * hh : 32 * hh + 32, :],
                                lhsT=ones[:],
                                rhs=ma[:, QT * hh : QT * hh + QT],
                                start=False,
                                stop=(ck == NCHUNK - 1 and hh == 3),
                                skip_group_check=True,
                                tile_position=(0, 32 * hh),
                            )
                    rc = rpool.tile([128, QT], f32, tag="rc", name="rc")
                    nc.vector.reciprocal(rc[:], sm[:])
                    nc.vector.tensor_mul(
                        oT[g][:, QT * qt : QT * qt + QT], pv[:], rc[:]
                    )

            # ---- phase 4: out proj + residual ----
            res = [cpool.tile([128, NQ], bft, tag=f"res{t}", name=f"res{t}") for t in range(2)]
            for oc in range(2):
                for nt in range(3):
                    n0 = QT * nt
                    ps = psB.tile([128, 512], f32, tag="ps", name="ps")
                    for cc in range(2):
                        nc.tensor.matmul(
                            ps[:, :QT],
                            lhsT=wo[cc][:, 128 * oc : 128 * oc + 128],
                            rhs=oT[cc][:, n0 : n0 + QT],
                            start=(cc == 0),
                            stop=(cc == 1),
                        )
                    nc.vector.tensor_add(
                        res[oc][:, n0 : n0 + QT],
                        ps[:, :QT],
                        xres[oc][:, n0 : n0 + QT],
                    )

            # ---- phase 5: 1x1 conv + bias + relu -> out ----
            for oc in range(2):
                for nt in range(3):
                    n0 = QT * nt
                    ps = psB.tile([128, 512], f32, tag="ps", name="ps")
                    for cc in range(2):
                        nc.tensor.matmul(
                            ps[:, :QT],
                            lhsT=wc[cc][:, 128 * oc : 128 * oc + 128],
                            rhs=res[cc][:, n0 : n0 + QT],
                            start=(cc == 0),
                            stop=(cc == 1),
                        )
                    ob = opool.tile([128, QT], f32, tag="ob", name="ob")
                    nc.vector.tensor_scalar(
                        ob[:],
                        ps[:, :QT],
                        bcr[:, oc : oc + 1],
                        0.0,
                        OP.add,
                        OP.max,
                    )
                    nc.sync.dma_start(
                        out_d[128 * oc : 128 * oc + 128, n0 : n0 + QT], ob[:]
                    )

    nc.compile()
    return nc


def _get_program():
    global _PROG
    if _PROG is None:
        _PROG = _build_program()
    return _PROG


def _prep_core_inputs(core, x, in_proj_w, in_proj_b, out_w, out_b, conv_w, conv_b):
    b, half = core // 2, core % 2
    roll = 8 - ROWS_HALF * half  # shift so query rows land at [8, 32)
    ximg = x[b].reshape(C, H, W)
    xroll = np.roll(ximg, roll, axis=1).reshape(C, N)
    # residual input: true query rows + out_b, fp32 (un-rolled half rows)
    xres = (
        ximg[:, ROWS_HALF * half : ROWS_HALF * half + ROWS_HALF, :].reshape(C, NQ)
        + out_b[:, None]
    ).astype(np.float32)
    return {
        "xT": xroll.astype(bf16),
        "xres": xres,
        "wqkT": np.ascontiguousarray(in_proj_w[: 2 * C].T).astype(bf16),
        "wvT": np.ascontiguousarray(in_proj_w[2 * C :].T).astype(bf16),
        "woT": np.ascontiguousarray(out_w.T).astype(bf16),
        "wcT": np.ascontiguousarray(conv_w.T).astype(bf16),
        "bqk": np.ascontiguousarray(
            in_proj_b[: 2 * C].reshape(4, 128).T
        ).astype(np.float32),
        "bvrep": np.broadcast_to(in_proj_b[2 * C :], (128, C)).astype(np.float32).copy(),
        "bcrep": np.ascontiguousarray(conv_b.reshape(2, 128).T).astype(np.float32),
        "masks": _rolled_masks(half),
    }


_MASK_CACHE = {}


def _rolled_masks(half: int) -> np.ndarray:
    """Masks in ROLLED coordinates.

    In rolled coords the query rows are [8, 32) and regions are
    [8*qt, 8*qt+24). A rolled row rr corresponds to true image row
    (rr - roll) mod 48 with roll = 8 - 24*half. The window test must use
    TRUE rows (and kill wrapped neighbors), so build the mask from true
    coordinates of both query and key tokens.
    """
    if half in _MASK_CACHE:
        return _MASK_CACHE[half]
    roll = 8 - ROWS_HALF * half
    out = np.zeros((3 * NCHUNK, 128, QT), np.float32)
    for qt in range(3):
        r0 = 8 + 8 * qt
        rs = 8 * qt
        qidx = r0 * W + np.arange(QT)
        qh_roll, qw = qidx // W, qidx % W
        qh = (qh_roll - roll) % H
        for ck in range(NCHUNK):
            kidx = rs * W + 128 * ck + np.arange(128)
            kh_roll, kw = kidx // W, kidx % W
            kh = (kh_roll - roll) % H
            # true-row distance; wrapped rows land far away and are masked
            m = (np.abs(kh[:, None] - qh[None, :]) <= HALF) & (
                np.abs(kw[:, None] - qw[None, :]) <= HALF
            )
            out[qt * NCHUNK + ck] = m
    res = out.astype(bf16)
    _MASK_CACHE[half] = res
    return res


def kernel(**inputs):
    from concourse.bass_utils import run_bass_kernel_spmd

    args = {k: np.asarray(v) for k, v in inputs.items()}
    nc = _get_program()
    in_maps = [
        _prep_core_inputs(core, **args) for core in range(NCORES)
    ]
    res = run_bass_kernel_spmd(nc, in_maps, core_ids=list(range(NCORES)))
    out = np.zeros((B, C, H, W), np.float32)
    for core in range(NCORES):
        b, half = core // 2, core % 2
        o = res.results[core]["out"].reshape(C, ROWS_HALF, W)
        out[b][:, ROWS_HALF * half : ROWS_HALF * half + ROWS_HALF, :] = o
    return out


# revision 22
# speedup vs baseline: 1.5569x; 1.5569x over previous
"""Trainium2 Bass kernel for ChunkedLocalSelfAttention.

Module: x[B,C,H,W] -> qkv proj -> 8-head local-window attention (17x17
spatial window) -> out proj -> +residual -> 1x1 conv -> relu.
B,C,H,W = 4,256,48,48; N = 2304 tokens per image; head dim 32.

Sharding: 8 cores = 4 batch images x 2 query-row-halves (24 rows each).
Each core computes the full pipeline for its half-image: attention output
rows only depend on +-8 image rows, so cores need no communication; the
row halo is covered by computing k/v for a 32-row band.

On-core design (scores kept TRANSPOSED: keys on partitions, queries free):
  - qk projection: qkT [512, 2304] = WqkT.T @ xT, bf16
  - v in [token, channel] layout: v = xT.T @ WvT for the 32-row band
  - per query-tile (8 image rows = 384 queries) and head-group (4 heads):
      for each of 9 key-chunks (128 contiguous tokens of the 24-row region):
        scoresT[k=128, h*512+q] via 4 row-packed K=32 matmuls
        exp on ScalarE (scale=1/sqrt(32) fused; logits are tiny so no
        max-subtraction is needed), multiplicative binary window mask on
        VectorE (mask precomputed on host, shared across heads)
        PV += v_chunk.T @ masked  (4 col-packed matmuls, M=32)
        sums += ones.T @ masked   (4 col-packed matmuls, M=32, the ones
                                   lhsT replicates each head's sum into
                                   its 32-partition strip)
      oT = PV * reciprocal(sums) -> bf16
  - out proj, +residual(+out_b folded on host), 1x1 conv, relu+conv_b.
"""

import sys

for _p in ("/opt/trn_rl_repo",):
    if _p not in sys.path:
        sys.path.insert(0, _p)

import math

import ml_dtypes
import numpy as np

B, C, H, W = 4, 256, 48, 48
N = H * W
HEADS, HD, HALF = 8, 32, 8
NCORES = 8
ROWS_HALF = H // 2          # 24 query rows per core
NQ = ROWS_HALF * W          # 1152 queries per core
REG_ROWS = 24               # key-region rows per query tile
NK = REG_ROWS * W           # 1152 keys per region = 9 chunks of 128
NCHUNK = NK // 128          # 9
BAND_ROWS = 32              # k/v row band per core (24 + 8 halo)
QT = 384                    # queries per tile (8 image rows)
SCALE = 1.0 / math.sqrt(HD)

bf16 = ml_dtypes.bfloat16

_PROG = None


def _build_program():
    import concourse.bass as bass
    import concourse.mybir as mybir
    import concourse.tile as tile
    from concourse import bacc

    f32 = mybir.dt.float32
    bft = mybir.dt.bfloat16
    AF = mybir.ActivationFunctionType
    OP = mybir.AluOpType

    nc = bacc.Bacc(
        "TRN2", target_bir_lowering=False, debug=False, num_devices=NCORES
    )

    def din(name, shape, dt=bft):
        return nc.dram_tensor(name, shape, dt, kind="ExternalInput").ap()

    xt_d = din("xT", [C, N])
    xres_d = din("xres", [C, NQ], f32)
    wqk_d = din("wqkT", [C, 2 * C])
    wv_d = din("wvT", [C, C])
    wo_d = din("woT", [C, C])
    wc_d = din("wcT", [C, C])
    bqk_d = din("bqk", [128, 4], f32)
    bv_d = din("bvrep", [128, C], f32)
    bc_d = din("bcrep", [128, 2], f32)
    mask_d = din("masks", [24, 128, QT])
    out_d = nc.dram_tensor("out", [C, NQ], f32, kind="ExternalOutput").ap()

    # SPMD trick: one program must serve both row-halves. The host ships
    # half-1 images VERTICALLY FLIPPED (attention is equivariant under a
    # row flip; the window test is |dh|<=8), so every core sees half-0
    # geometry: query rows [0, 24), key band rows [0, 32). Query tile qt
    # has rows [8qt, 8qt+8) and key-region rows [rs, rs+24), rs={0,0,8}.
    # For qt=0 the last 3 region chunks (rows 16-24) are fully outside the
    # +-8 row window of its queries and are skipped entirely.

    with tile.TileContext(nc) as tc:
        import contextlib

        ctx = contextlib.ExitStack()
        with ctx:
            cpool = ctx.enter_context(tc.tile_pool(name="const", bufs=1))
            qkpool = ctx.enter_context(tc.tile_pool(name="qk", bufs=1))
            vpool = ctx.enter_context(tc.tile_pool(name="v", bufs=1))
            epool = ctx.enter_context(tc.tile_pool(name="exp", bufs=4))
            apool = ctx.enter_context(tc.tile_pool(name="attn", bufs=4))
            rpool = ctx.enter_context(tc.tile_pool(name="recip", bufs=3))
            opool = ctx.enter_context(tc.tile_pool(name="outb", bufs=3))
            psA = ctx.enter_context(
                tc.tile_pool(name="psA", bufs=2, space="PSUM")
            )
            psB = ctx.enter_context(
                tc.tile_pool(name="psB", bufs=2, space="PSUM")
            )

            # ---- constants / inputs to SBUF ----
            xt = [cpool.tile([128, N], bft, tag=f"xt{t}", name=f"xt{t}") for t in range(2)]
            for t in range(2):
                nc.sync.dma_start(xt[t][:], xt_d[128 * t : 128 * t + 128, :])
            wqk = [cpool.tile([128, 2 * C], bft, tag=f"wqk{t}", name=f"wqk{t}") for t in range(2)]
            wv = [cpool.tile([128, C], bft, tag=f"wv{t}", name=f"wv{t}") for t in range(2)]
            wo = [cpool.tile([128, C], bft, tag=f"wo{t}", name=f"wo{t}") for t in range(2)]
            wc = [cpool.tile([128, C], bft, tag=f"wc{t}", name=f"wc{t}") for t in range(2)]
            for t in range(2):
                sl = slice(128 * t, 128 * t + 128)
                nc.sync.dma_start(wqk[t][:], wqk_d[sl, :])
                nc.sync.dma_start(wv[t][:], wv_d[sl, :])
                nc.sync.dma_start(wo[t][:], wo_d[sl, :])
                nc.sync.dma_start(wc[t][:], wc_d[sl, :])
            bqk = cpool.tile([128, 4], f32, tag="bqk")
            bvr = cpool.tile([128, C], f32, tag="bvr")
            bcr = cpool.tile([128, 2], f32, tag="bcr")
            nc.sync.dma_start(bqk[:], bqk_d[:])
            nc.sync.dma_start(bvr[:], bv_d[:])
            nc.sync.dma_start(bcr[:], bc_d[:])
            zrow = cpool.tile([1, 512], bft, tag="zrow")
            nc.vector.memset(zrow[:], 0.0)
            msk = cpool.tile([128, 24 * QT], bft, tag="msk")
            nc.sync.dma_start(
                msk[:].rearrange("p (c q) -> p c q", q=QT),
                mask_d[:].transpose([1, 0, 2]),
            )
            xres = [cpool.tile([128, NQ], f32, tag=f"xres{t}", name=f"xres{t}") for t in range(2)]
            for t in range(2):
                nc.sync.dma_start(xres[t][:], xres_d[128 * t : 128 * t + 128, :])

            # ---- phase 1: qk projection  qkT[512, N] bf16 ----
            # q needed for tokens [0, 1152) only; k for the band [0, 1536)
            qk = [qkpool.tile([128, 1536], bft, tag=f"qk{i}", name=f"qk{i}") for i in range(4)]
            NT_Q = [(0, 384), (384, 384), (768, 384)]
            NT_K = [(0, 512), (512, 512), (1024, 512)]

            def qk_proj(qc):
                for n0, nw in (NT_Q if qc < 2 else NT_K):
                    ps = psB.tile([128, 512], f32, tag="ps", name="ps")
                    for cc in range(2):
                        nc.tensor.matmul(
                            ps[:, :nw],
                            lhsT=wqk[cc][:, 128 * qc : 128 * qc + 128],
                            rhs=xt[cc][:, n0 : n0 + nw],
                            start=(cc == 0),
                            stop=(cc == 1),
                        )
                    nc.vector.tensor_scalar_add(
                        qk[qc][:, n0 : n0 + nw], ps[:, :nw], bqk[:, qc : qc + 1]
                    )

            # v band, token-major, rows [0, 32) -> 12 tiles; layout per
            # tile: head h cols [64h, 64h+32) = v_h, [64h+32, 64h+64) = 1.0
            vt = [vpool.tile([128, 8 * 64], bft, tag=f"v{i}", name=f"v{i}") for i in range(12)]

            def v_proj(i):
                n0 = 128 * i
                ps = psB.tile([128, 512], f32, tag="ps", name="ps")
                for cc in range(2):
                    nc.tensor.matmul(
                        ps[:, :C],
                        lhsT=xt[cc][:, n0 : n0 + 128],
                        rhs=wv[cc][:],
                        start=(cc == 0),
                        stop=(cc == 1),
                    )
                va = vt[i][:].rearrange("p (h two v) -> p h two v", two=2, v=32)
                nc.vector.tensor_add(
                    va[:, :, 0, :],
                    ps[:, :C].rearrange("p (h v) -> p h v", v=32),
                    bvr[:].rearrange("p (h v) -> p h v", v=32),
                )
                nc.gpsimd.memset(va[:, :, 1, :], 1.0)

            # heads 0-3 inputs first so attention can start early
            qk_proj(0)
            qk_proj(2)
            for i in range(6):
                v_proj(i)
            qk_proj(1)
            qk_proj(3)
            for i in range(6, 12):
                v_proj(i)

            # ---- phase 3: attention ----
            # rolled coords: query rows [8, 32): qtile qt rows r0 = 8+8*qt,
            # region rows rs = 8*qt, region tokens [rs*48, rs*48+1152).
            oT = [cpool.tile([128, NQ], bft, tag=f"oT{g}", name=f"oT{g}") for g in range(2)]
            res = [cpool.tile([128, NQ], bft, tag=f"res{t}", name=f"res{t}") for t in range(2)]
            mbase = 0
            for qt in range(3):
                r0 = 8 * qt
                rs = (0, 0, 8)[qt]
                nchunk = (6, 9, 9)[qt]
                q0 = r0 * W
                for g in range(4):
                    # pair tile pp: rows = [pv_h | sums_h | pv_h' | sums_h']
                    # for heads (2g, 2g+1). Col-packed accumulation: two
                    # M=64 matmuls share the bank, so start=True (bank-wide
                    # has_written clear) is unusable; zero the bank and
                    # accumulate from the first matmul.
                    # zero-matmul opens the accumulation group: start=True
                    # clears the bank's has_written and writes zeros to every
                    # element, so the PV matmuls below accumulate from zero.
                    pp = psB.tile([128, QT], f32, tag="pp", name="pp", bufs=2)
                    nc.tensor.matmul(
                        pp[:],
                        lhsT=zrow[:, 0:128],
                        rhs=zrow[:, 0:QT],
                        start=True,
                        stop=False,
                        skip_group_check=True,
                    )
                    for ck in range(nchunk):
                        kof = rs * W + 128 * ck
                        # only queries within +-8 rows of this chunk's keys
                        # participate; the range is 128-token aligned.
                        a = max(kof - 384 - QT * qt, 0)
                        b = min(kof + 512 - QT * qt, QT)
                        qw_ = b - a
                        sc = psA.tile([128, 1024], f32, tag="sc", name="sc")
                        for hh in range(2):
                            h = 2 * g + hh
                            qtile_idx, krow = h // 4, 32 * (h % 4)
                            nc.tensor.matmul(
                                sc[:, 512 * hh + a : 512 * hh + b],
                                lhsT=qk[2 + qtile_idx][
                                    krow : krow + 32, kof : kof + 128
                                ],
                                rhs=qk[qtile_idx][
                                    krow : krow + 32, q0 + a : q0 + b
                                ],
                                start=True,
                                stop=True,
                                tile_position=(krow, 0),
                            )
                        ex = epool.tile([128, 2 * QT], bft, tag="ex", name="ex")
                        sc_v = sc[:].rearrange("p (h q) -> p h q", q=512)[
                            :, :, a:b
                        ]
                        ex_v = ex[:].rearrange("p (h q) -> p h q", q=QT)[
                            :, :, a:b
                        ]
                        nc.scalar.activation(ex_v, sc_v, AF.Exp, scale=SCALE)
                        ma = apool.tile([128, 2 * QT], bft, tag="ma", name="ma")
                        ma_v = ma[:].rearrange("p (h q) -> p h q", q=QT)[
                            :, :, a:b
                        ]
                        mk = msk[:, (mbase + ck) * QT + a : (mbase + ck) * QT + b]
                        nc.vector.tensor_mul(
                            ma_v, ex_v, mk[:, None, :].broadcast_to([128, 2, qw_])
                        )
                        vi = vt[(rs * W + 128 * ck) // 128]
                        for hh in range(2):
                            h = 2 * g + hh
                            nc.tensor.matmul(
                                pp[64 * hh : 64 * hh + 64, a:b],
                                lhsT=vi[:, 64 * h : 64 * h + 64],
                                rhs=ma[:, QT * hh + a : QT * hh + b],
                                start=False,
                                stop=(ck == nchunk - 1 and hh == 1),
                                skip_group_check=True,
                                tile_position=(0, 64 * hh),
                            )
                    # rows of pp: 0-31 pv_a, 32-63 sums_a, 64-95 pv_b, 96-127 sums_b
                    rc = rpool.tile([128, QT], f32, tag="rc", name="rc")
                    nc.vector.reciprocal(rc[:], pp[:])
                    # shift recip(sums) down 32 partitions onto pv lanes
                    rcs = rpool.tile([128, QT], f32, tag="rcs", name="rcs")
                    nc.sync.dma_start(rcs[0:96, :], rc[32:128, :])
                    on = rpool.tile([128, QT], bft, tag="on", name="on")
                    nc.vector.tensor_mul(on[0:96, :], pp[0:96, :], rcs[0:96, :])
                    # compact pv rows {0-31, 64-95} into channel order
                    nc.sync.dma_start(
                        oT[g // 2][
                            64 * (g % 2) : 64 * (g % 2) + 32,
                            QT * qt : QT * qt + QT,
                        ],
                        on[0:32, :],
                    )
                    nc.sync.dma_start(
                        oT[g // 2][
                            64 * (g % 2) + 32 : 64 * (g % 2) + 64,
                            QT * qt : QT * qt + QT,
                        ],
                        on[64:96, :],
                    )
                # ---- projections for this qtile's columns ----
                n0 = QT * qt
                for oc in range(2):
                    ps = psB.tile([128, 512], f32, tag="ps", name="ps")
                    for cc in range(2):
                        nc.tensor.matmul(
                            ps[:, :QT],
                            lhsT=wo[cc][:, 128 * oc : 128 * oc + 128],
                            rhs=oT[cc][:, n0 : n0 + QT],
                            start=(cc == 0),
                            stop=(cc == 1),
                        )
                    nc.vector.tensor_add(
                        res[oc][:, n0 : n0 + QT],
                        ps[:, :QT],
                        xres[oc][:, n0 : n0 + QT],
                    )
                for oc in range(2):
                    ps = psB.tile([128, 512], f32, tag="ps", name="ps")
                    for cc in range(2):
                        nc.tensor.matmul(
                            ps[:, :QT],
                            lhsT=wc[cc][:, 128 * oc : 128 * oc + 128],
                            rhs=res[cc][:, n0 : n0 + QT],
                            start=(cc == 0),
                            stop=(cc == 1),
                        )
                    ob = opool.tile([128, QT], f32, tag="ob", name="ob")
                    nc.vector.tensor_scalar(
                        ob[:],
                        ps[:, :QT],
                        bcr[:, oc : oc + 1],
                        0.0,
                        OP.add,
                        OP.max,
                    )
                    nc.sync.dma_start(
                        out_d[128 * oc : 128 * oc + 128, n0 : n0 + QT], ob[:]
                    )
                mbase += nchunk

    nc.compile()
    return nc


def _get_program():
    global _PROG
    if _PROG is None:
        _PROG = _build_program()
    return _PROG


def _prep_core_inputs(core, x, in_proj_w, in_proj_b, out_w, out_b, conv_w, conv_b):
    b, half = core // 2, core % 2
    ximg = x[b].reshape(C, H, W)
    if half == 1:
        ximg = ximg[:, ::-1, :]  # row-flip: half-1 becomes half-0 geometry
    xres = (ximg[:, :ROWS_HALF, :].reshape(C, NQ) + out_b[:, None]).astype(
        np.float32
    )
    return {
        "xT": np.ascontiguousarray(ximg.reshape(C, N)).astype(bf16),
        "xres": xres,
        "wqkT": np.ascontiguousarray(in_proj_w[: 2 * C].T).astype(bf16),
        "wvT": np.ascontiguousarray(in_proj_w[2 * C :].T).astype(bf16),
        "woT": np.ascontiguousarray(out_w.T).astype(bf16),
        "wcT": np.ascontiguousarray(conv_w.T).astype(bf16),
        "bqk": np.ascontiguousarray(
            in_proj_b[: 2 * C].reshape(4, 128).T
        ).astype(np.float32),
        "bvrep": np.broadcast_to(in_proj_b[2 * C :], (128, C)).astype(np.float32).copy(),
        "bcrep": np.ascontiguousarray(conv_b.reshape(2, 128).T).astype(np.float32),
        "masks": _masks(),
    }


_MASK_CACHE = {}


def _masks() -> np.ndarray:
    """[24, 128, 384] binary window masks, shared by every core.

    Half-0 geometry: qtile qt queries rows [8qt, 8qt+8), region rows
    [rs, rs+24) with rs = (0, 0, 8)[qt]; qt=0 keeps only chunks 0-5.
    """
    if "m" in _MASK_CACHE:
        return _MASK_CACHE["m"]
    outs = []
    for qt, (rs, nchunk) in enumerate(zip((0, 0, 8), (6, 9, 9))):
        r0 = 8 * qt
        qidx = r0 * W + np.arange(QT)
        qh, qw = qidx // W, qidx % W
        for ck in range(nchunk):
            kidx = rs * W + 128 * ck + np.arange(128)
            kh, kw = kidx // W, kidx % W
            m = (np.abs(kh[:, None] - qh[None, :]) <= HALF) & (
                np.abs(kw[:, None] - qw[None, :]) <= HALF
            )
            outs.append(m)
    res = np.stack(outs).astype(bf16)
    _MASK_CACHE["m"] = res
    return res


def kernel(**inputs):
    from concourse.bass_utils import run_bass_kernel_spmd

    args = {k: np.asarray(v) for k, v in inputs.items()}
    nc = _get_program()
    in_maps = [
        _prep_core_inputs(core, **args) for core in range(NCORES)
    ]
    res = run_bass_kernel_spmd(nc, in_maps, core_ids=list(range(NCORES)))
    out = np.zeros((B, C, H, W), np.float32)
    for core in range(NCORES):
        b, half = core // 2, core % 2
        o = res.results[core]["out"].reshape(C, ROWS_HALF, W)
        if half == 1:
            o = o[:, ::-1, :]  # undo the row flip
            out[b][:, ROWS_HALF:, :] = o
        else:
            out[b][:, :ROWS_HALF, :] = o
    return out
